# revision 33
# baseline (speedup 1.0000x reference)
"""Trainium2 Bass kernel for nn_CacheModel (retrieval_knn).

Computes out = log(exp(theta * (x/||x||) @ mem_keys) @ mem_vals) on 8
NeuronCores.  mem_keys is sharded column-wise and mem_vals row-wise over
the N_mem axis; each core computes its partial [1,1000] product, an
on-device AllReduce sums the partials, and each core takes the log.

This problem is HBM-bound (per core: keys 51MB + vals 25MB must stream
through once), so everything rides on bytes-per-element and DMA
efficiency:

* keys, vals AND the exp() intermediates are all fp8 e4m3 (TRN FP8_EXP4,
  max 240 — ml_dtypes.float8_e4m3 matches bit-for-bit).  The log output
  only needs ~2e-2 relative accuracy, and a numpy bit-exact forecast of
  this quantization measures 1.9e-3: the exponent error from e4m3 keys
  (~0.13 rms) dominates and the log turns p-space error back into small
  absolute error.
* exp() has ~e^22 dynamic range, far past fp8.  A global shift C=17 is
  applied inside the activation (exp(s*theta/||x|| - 17)) and folded
  back into the final log via its input scale e^C; p_mem is only needed
  up to a scale.  The shift is safe for this input distribution
  (5*s_max ~ 21.9 -> max exp value ~135 < e4m3's 240) and entries below
  e4m3's subnormal floor contribute < 1e-3 relatively.
* both matmul stages run in fp8 DoubleRow perf mode (2 fp8 weights per
  PE cell, K=256 per matmul), halving PE streaming time so the tensor
  engine stays off the critical path.
* keys+vals for one 512-row window ship as a single fused [128, 12192]
  fp8 DMA (1.56 MB) from a contiguous DRAM block.
* a dummy [1,1] AllReduce in the prologue absorbs the ~11us
  collective-core startup latency that would otherwise sit on the tail,
  and the Ln activation table is preloaded for the same reason.

x ships as fp32 and is split on-device into an fp8 (hi, lo*16) pair
used as an M=2 DoubleRow stationary; the hi/lo recombination runs as a
bf16 [2,128]^T @ [1, 1/16] transpose matmul (bf16 weights take the fast
FWL load path), and the theta/||x|| scale is applied at fp32 precision
inside the Exp activation.

The tensor engine idles part of each window in this DMA-bound pipeline,
so the HAM clock gate oscillates between 1.2 and 2.4 GHz; keeping the
per-window PE work well under the DMA period (DoubleRow + bf16
transposes) makes the cold phases harmless.  Measured: ~250us of
compute per core (DMA busy ~94% of the chip-aggregate HBM roofline when
all 8 cores run together), plus ~20us collective tail, plus whatever
launch skew the runtime draws that run (0-165us observed).

Self-contained: hardcodes all shapes; imports only the system-installed
concourse stack + numpy.
"""

from contextlib import ExitStack

import ml_dtypes
import numpy as np

import concourse.bass as bass
import concourse.tile as tile
from concourse import bacc, mybir

F32 = mybir.dt.float32
BF16 = mybir.dt.bfloat16
F8 = mybir.dt.float8e4
AF = mybir.ActivationFunctionType
DR = mybir.MatmulPerfMode.DoubleRow
F8_NP = ml_dtypes.float8_e4m3  # TRN FP8_EXP4-compatible (max 240)

# Problem shapes (full)
D_FEAT = 2048
N_MEM = 200000
N_CLASSES = 1000
THETA = 5.0
N_CORES = 8

# Per-core sharding: 25000 n-rows, zero-padded to 25088 = 49*512
N_SHARD = N_MEM // N_CORES          # 25000
WIN = 512                           # n-window width (one psum bank of f32)
N_PAD = 25088                       # 49 windows * 512
N_WINDOWS = N_PAD // WIN            # 49
CHUNKS_PER_WIN = WIN // 128         # 4
FEAT_CHUNKS = D_FEAT // 128         # 16
NC_HALF = N_CLASSES // 2            # 500 (<=512 psum free-dim limit)
KEY_BYTES = FEAT_CHUNKS * WIN       # 8192 per partition per window
VAL_BYTES = CHUNKS_PER_WIN * N_CLASSES  # 4000 per partition per window
KV_BYTES = KEY_BYTES + VAL_BYTES    # 12192
C_SHIFT = 17.0                      # global exp shift; added back post-log
XLO_SCALE = 16.0                    # x-lo residual premultiplier


def build_kernel(num_devices: int = N_CORES, kv_bufs: int = 8):
    """Builds + compiles the per-core Bass program (SPMD: same program on
    every core; each core receives its own fused keys/vals shard)."""
    nc = bacc.Bacc(
        "TRN2",
        target_bir_lowering=False,
        debug=False,
        num_devices=num_devices,
    )

    x_d = nc.dram_tensor("x", [1, D_FEAT], F32, kind="ExternalInput").ap()
    # Fused keys+vals blocks, two windows per contiguous DMA (3.1 MB each;
    # per-partition runs stay fully contiguous so DMA sustains line rate):
    #   window block w at pair t=w//2, half i=w%2:
    #     kv[t, p, i*KVB + c*WIN + j]        = e4m3(keys[c*128+p, w*WIN+j])
    #     kv[t, p, i*KVB + KEYB + q*NCLS+j]  = e4m3(vals[(w*4+q)*128+p, j])
    n_pairs = (N_WINDOWS + 1) // 2
    kv_d = nc.dram_tensor(
        "kv", [n_pairs, 128, 2 * KV_BYTES], F8, kind="ExternalInput"
    ).ap()
    out_d = nc.dram_tensor("out", [1, N_CLASSES], F32, kind="ExternalOutput").ap()

    with tile.TileContext(nc) as tc, ExitStack() as ctx:
        const = ctx.enter_context(tc.tile_pool(name="const", bufs=1))
        kv_pool = ctx.enter_context(tc.tile_pool(name="kv", bufs=kv_bufs))
        s_pool = ctx.enter_context(tc.tile_pool(name="s", bufs=4))
        ss_pool = ctx.enter_context(tc.tile_pool(name="ss", bufs=4))
        psum_s = ctx.enter_context(tc.tile_pool(name="psum_s", bufs=3, space="PSUM"))
        psum_t = ctx.enter_context(tc.tile_pool(name="psum_t", bufs=3, space="PSUM"))
        psum_p = ctx.enter_context(tc.tile_pool(name="psum_p", bufs=1, space="PSUM"))
        dram = ctx.enter_context(tc.tile_pool(name="dram", bufs=1, space="DRAM"))

        # ---- prologue.  Ordering is ramp-critical: the tiny x DMA goes
        # first on the sync queue (it gates the x fp8 split), window 0's kv
        # DMA right behind it; the x hi/lo split (the only stage-1
        # dependency) is emitted before the norm/scale chain, whose products
        # (scaleB/w2row/biasC) are not needed until the first post-chain two
        # windows later.
        xt = const.tile([128, FEAT_CHUNKS], F32)
        nc.sync.dma_start(out=xt[:], in_=x_d.rearrange("a (c p) -> p (a c)", p=128))

        kv0_t = kv_pool.tile([128, KV_BYTES], F8, tag="kv")
        nc.sync.dma_start(out=kv0_t[:], in_=kv_d[0, :, 0:KV_BYTES])

        # x fp8 hi/lo split (lo premultiplied by XLO_SCALE), laid out for
        # DoubleRow: xs[p, c, m], m in {hi, lo}; inner dim padded to 16 so
        # the k-pair stride is 16B (LDWEIGHTS DoubleRow AP constraint).
        xh8 = const.tile([128, FEAT_CHUNKS], F8)
        nc.vector.tensor_copy(xh8[:], xt[:])
        xh32 = const.tile([128, FEAT_CHUNKS], F32)
        nc.vector.tensor_copy(xh32[:], xh8[:])
        xl32 = const.tile([128, FEAT_CHUNKS], F32)
        nc.vector.tensor_sub(xl32[:], xt[:], xh32[:])
        xl16 = const.tile([128, FEAT_CHUNKS], F32)
        nc.vector.tensor_scalar_mul(xl16[:], xl32[:], XLO_SCALE)
        xs = const.tile([128, FEAT_CHUNKS, 16], F8)
        nc.vector.tensor_copy(
            xs[:, :, 0:1], xh8[:].rearrange("p (c o) -> p c o", o=1)
        )
        nc.vector.tensor_copy(
            xs[:, :, 1:2], xl16[:].rearrange("p (c o) -> p c o", o=1)
        )

        ones = const.tile([128, 1], F32)
        nc.vector.memset(ones[:], 1.0)

        sq = const.tile([128, FEAT_CHUNKS], F32)
        nc.vector.tensor_mul(sq[:], xt[:], xt[:])
        sums = const.tile([128, 1], F32)
        nc.vector.tensor_reduce(
            sums[:], sq[:], axis=mybir.AxisListType.X, op=mybir.AluOpType.add
        )
        nrm2_ps = psum_t.tile([1, 1], F32, tag="ps_t")
        nc.tensor.matmul(nrm2_ps[:], lhsT=ones[:], rhs=sums[:], start=True, stop=True)
        nrm = const.tile([1, 1], F32)
        nc.scalar.sqrt(nrm[:], nrm2_ps[:])
        inv = const.tile([1, 1], F32)
        nc.vector.reciprocal(inv[:], nrm[:])
        scale = const.tile([1, 1], F32)
        nc.vector.tensor_scalar_mul(scale[:], inv[:], THETA)
        # Preload the Ln activation table now; its first use otherwise sits
        # on the tail critical path as a 1.3us ACT_TABLE_LOAD.
        ln_warm = const.tile([1, 1], F32)
        nc.scalar.activation(ln_warm[:], nrm[:], AF.Ln)
        # w2row = [1, 1/16]^T in bf16 (exact): recombines the hi/lo planes in
        # the [2,128]^T @ w2row transpose matmul.  bf16 weights get the fast
        # FWL weight-load path; fp32 would cost ~4x there.  (Built via a tiny
        # transpose matmul — engines cannot memset at a partition offset.)
        w2f = const.tile([1, 2], F32)
        nc.vector.memset(w2f[:, 0:1], 1.0)
        nc.vector.memset(w2f[:, 1:2], 1.0 / XLO_SCALE)
        onep = const.tile([1, 1], F32)
        nc.vector.memset(onep[:], 1.0)
        w2_ps = psum_t.tile([2, 1], F32, tag="ps_t")
        nc.tensor.matmul(w2_ps[:], lhsT=w2f[:], rhs=onep[:], start=True, stop=True)
        w2row = const.tile([2, 1], BF16)
        nc.vector.tensor_copy(w2row[:], w2_ps[:])
        # scaleB = theta/||x|| broadcast to [128,1]; applied at full fp32
        # precision inside the Exp activation (out = exp(in*scale + bias)).
        ones_row = const.tile([1, 128], F32)
        nc.vector.memset(ones_row[:], 1.0)
        scaleB_ps = psum_t.tile([128, 1], F32, tag="ps_t")
        nc.tensor.matmul(scaleB_ps[:], lhsT=ones_row[:], rhs=scale[:], start=True, stop=True)
        scaleB = const.tile([128, 1], F32)
        nc.vector.tensor_copy(scaleB[:], scaleB_ps[:])
        biasC = const.tile([128, 1], F32)
        nc.vector.memset(biasC[:], -C_SHIFT)

        # Warm the collective-compute cores with a tiny dummy AllReduce now:
        # the first collective pays ~11us of CC startup latency before its
        # mesh begins, which otherwise lands on the tail critical path.
        warm_sb = const.tile([1, 1], F32)
        nc.vector.memset(warm_sb[:], 0.0)
        warm_in = dram.tile([1, 1], F32)
        warm_out = dram.tile([1, 1], F32)
        nc.gpsimd.dma_start(warm_in[:], warm_sb[:])
        nc.gpsimd.collective_compute(
            "AllReduce",
            mybir.AluOpType.add,
            replica_groups=[list(range(num_devices))],
            ins=[warm_in.opt()],
            outs=[warm_out.opt()],
        )

        # ---- persistent [1, NC_HALF] accumulators (class halves)
        pp_a = psum_p.tile([1, NC_HALF], F32, tag="pp_a")
        pp_b = psum_p.tile([1, NC_HALF], F32, tag="pp_b")

        n_groups = N_WINDOWS * 2  # stage-2 accumulation steps per class half

        def emit_post(ps_s, vals, w):
            s2 = s_pool.tile([2, WIN], BF16, tag="s2")
            nc.vector.tensor_copy(s2[:], ps_s[:])
            # ss[p, q, 0] = e4m3(exp(scale*(hi + lo/16) - C)): bf16 transpose
            # matmul recombines hi/lo, then Exp applies the fp32 scale and
            # bias and writes fp8 directly (max value ~134 < e4m3's 240).
            ss = ss_pool.tile([128, CHUNKS_PER_WIN, 16], F8, tag="ss")
            for q in range(CHUNKS_PER_WIN):
                ps_t = psum_t.tile([128, 1], F32, tag="ps_t")
                nc.tensor.matmul(
                    ps_t[:],
                    lhsT=s2[:, q * 128:(q + 1) * 128],
                    rhs=w2row[:],
                    start=True,
                    stop=True,
                )
                nc.scalar.activation(
                    ss[:, q, 0:1], ps_t[:], AF.Exp, bias=biasC[:], scale=scaleB[:]
                )
            # stage 2: fp8 DoubleRow, two n-chunk pairs x two class halves
            for r in range(2):
                gc = w * 2 + r
                first = gc == 0
                last = gc == n_groups - 1
                for pp, j0 in ((pp_a, 0), (pp_b, NC_HALF)):
                    nc.tensor.matmul(
                        pp[:],
                        lhsT=ss[:, 2 * r:2 * r + 2, 0:1],
                        rhs=vals[:, 2 * r:2 * r + 2, j0:j0 + NC_HALF],
                        start=first,
                        stop=last,
                        perf_mode=DR,
                        skip_group_check=True,
                    )

        # Software-pipelined emission, depth 2: window w's post-chain
        # (transpose/exp/stage-2) is emitted after window w+2's stage-1
        # matmuls, giving the ACT/DVE exp+cast chain a full extra window
        # to complete before the PE needs its stage-2 operands.  One
        # dma_start per window: 1.56 MB transfers sustain a higher measured
        # line rate than 3.1 MB ones, and per-window completion lets each
        # stage-1 start sooner.
        pends = []
        for w in range(N_WINDOWS):
            if w == 0:
                kv_t = kv0_t  # DMA'd at the top of the prologue
            else:
                kv_t = kv_pool.tile([128, KV_BYTES], F8, tag="kv")
                nc.sync.dma_start(
                    out=kv_t[:],
                    in_=kv_d[
                        w // 2, :, (w % 2) * KV_BYTES:(w % 2 + 1) * KV_BYTES
                    ],
                )
            keys = kv_t[:, 0:KEY_BYTES].rearrange(
                "p (c j) -> p c j", c=FEAT_CHUNKS
            )
            vals = kv_t[:, KEY_BYTES:KV_BYTES].rearrange(
                "p (q j) -> p q j", q=CHUNKS_PER_WIN
            )
            # stage 1: fp8 DoubleRow, 8 matmuls of K=256 each
            ps_s = psum_s.tile([2, WIN], F32)
            for c in range(FEAT_CHUNKS // 2):
                nc.tensor.matmul(
                    ps_s[:],
                    lhsT=xs[:, 2 * c:2 * c + 2, 0:2],
                    rhs=keys[:, 2 * c:2 * c + 2, :],
                    start=(c == 0),
                    stop=(c == FEAT_CHUNKS // 2 - 1),
                    perf_mode=DR,
                    skip_group_check=True,
                )

            pends.append((ps_s, vals, w))
            if len(pends) > 2:
                emit_post(*pends.pop(0))
        for p in pends:
            emit_post(*p)

        # ---- tail: partial p = [pp_a | pp_b]; AllReduce; log; +C
        p_sb = const.tile([1, N_CLASSES], F32)
        nc.vector.tensor_copy(p_sb[:, 0:NC_HALF], pp_a[:])
        nc.vector.tensor_copy(p_sb[:, NC_HALF:N_CLASSES], pp_b[:])

        partial = dram.tile([1, N_CLASSES], F32)
        reduced = dram.tile([1, N_CLASSES], F32)
        nc.sync.dma_start(partial[:], p_sb[:])
        nc.gpsimd.collective_compute(
            "AllReduce",
            mybir.AluOpType.add,
            replica_groups=[list(range(num_devices))],
            ins=[partial.opt()],
            outs=[reduced.opt()],
        )
        # Read the reduced partial back spread over 8 partitions, and fold
        # the +C un-shift into the Ln via its input scale:
        # ln(p*e^-C * e^C) = ln(p).
        red_sb = const.tile([8, N_CLASSES // 8], F32)
        nc.sync.dma_start(
            red_sb[:], reduced[:].rearrange("a (p j) -> p (a j)", p=8)
        )
        lg = const.tile([8, N_CLASSES // 8], F32)
        nc.scalar.activation(
            lg[:], red_sb[:], AF.Ln, scale=float(np.exp(C_SHIFT))
        )
        nc.sync.dma_start(out_d.rearrange("a (p j) -> p (a j)", p=8), lg[:])

    nc.compile()
    return nc


_NC_CACHE: dict = {}


def _get_nc():
    if "nc" not in _NC_CACHE:
        _NC_CACHE["nc"] = build_kernel()
    return _NC_CACHE["nc"]


def _retile_keys(keys_shard):
    """[D_FEAT, N_PAD] e4m3 -> [N_WINDOWS, 128, KEY_BYTES] with
    out[w, p, c*WIN + j] = keys_shard[c*128 + p, w*WIN + j]."""
    v = keys_shard.reshape(FEAT_CHUNKS, 128, N_WINDOWS, WIN)
    return np.ascontiguousarray(v.transpose(2, 1, 0, 3)).reshape(
        N_WINDOWS, 128, KEY_BYTES
    )


def _retile_vals(vals_shard):
    """[N_PAD, N_CLASSES] e4m3 -> [N_WINDOWS, 128, VAL_BYTES] with
    out[w, p, q*NCLS + j] = vals_shard[(w*4 + q)*128 + p, j]."""
    v = vals_shard.reshape(N_WINDOWS, CHUNKS_PER_WIN, 128, N_CLASSES)
    return np.ascontiguousarray(v.transpose(0, 2, 1, 3)).reshape(
        N_WINDOWS, 128, VAL_BYTES
    )


def _shard_inputs(x, mem_keys, mem_vals):
    x = np.ascontiguousarray(np.asarray(x, dtype=np.float32))
    keys8 = np.asarray(mem_keys, dtype=np.float32).astype(F8_NP)
    vals8 = np.asarray(mem_vals, dtype=np.float32).astype(F8_NP)
    n_pairs = (N_WINDOWS + 1) // 2
    in_maps = []
    for i in range(N_CORES):
        lo_i, hi_i = i * N_SHARD, (i + 1) * N_SHARD
        keys_shard = np.zeros((D_FEAT, N_PAD), dtype=F8_NP)
        keys_shard[:, :N_SHARD] = keys8[:, lo_i:hi_i]
        vals_shard = np.zeros((N_PAD, N_CLASSES), dtype=F8_NP)
        vals_shard[:N_SHARD, :] = vals8[lo_i:hi_i, :]
        kv = np.concatenate(
            [_retile_keys(keys_shard), _retile_vals(vals_shard)], axis=2
        )
        # pair consecutive windows contiguously: kv2[t, p, i*KVB + b]
        kv2 = np.zeros((n_pairs, 128, 2 * KV_BYTES), dtype=F8_NP)
        kv2[: N_WINDOWS // 2] = (
            kv[: N_WINDOWS // 2 * 2]
            .reshape(N_WINDOWS // 2, 2, 128, KV_BYTES)
            .transpose(0, 2, 1, 3)
            .reshape(N_WINDOWS // 2, 128, 2 * KV_BYTES)
        )
        if N_WINDOWS % 2:
            kv2[-1, :, 0:KV_BYTES] = kv[-1]
        in_maps.append({"x": x, "kv": np.ascontiguousarray(kv2)})
    return in_maps


def run(x, mem_keys, mem_vals, trace: bool = False):
    """Runs the SPMD kernel; returns (output [1, N_CLASSES], BassKernelResults)."""
    from concourse.bass_utils import run_bass_kernel_spmd

    nc = _get_nc()
    in_maps = _shard_inputs(x, mem_keys, mem_vals)
    res = run_bass_kernel_spmd(nc, in_maps, list(range(N_CORES)), trace=trace)
    out = np.asarray(res.results[0]["out"], dtype=np.float32).reshape(1, N_CLASSES)
    return out, res


def kernel(x, mem_keys, mem_vals):
    out, _ = run(x, mem_keys, mem_vals, trace=False)
    return out


# revision 34
# speedup vs baseline: 1.0737x; 1.0737x over previous
"""Trainium2 Bass kernel for nn_CacheModel (retrieval_knn).

Computes out = log(exp(theta * (x/||x||) @ mem_keys) @ mem_vals) on 8
NeuronCores.  mem_keys is sharded column-wise and mem_vals row-wise over
the N_mem axis; each core computes its partial [1,1000] product, an
on-device AllReduce sums the partials, and each core takes the log.

This problem is HBM-bound (per core: keys 51MB + vals 25MB must stream
through once), so everything rides on bytes-per-element and DMA
efficiency:

* keys, vals AND the exp() intermediates are all fp8 e4m3 (TRN FP8_EXP4,
  max 240 — ml_dtypes.float8_e4m3 matches bit-for-bit).  The log output
  only needs ~2e-2 relative accuracy, and a numpy bit-exact forecast of
  this quantization measures 1.9e-3: the exponent error from e4m3 keys
  (~0.13 rms) dominates and the log turns p-space error back into small
  absolute error.
* exp() has ~e^22 dynamic range, far past fp8.  A global shift C=17 is
  applied inside the activation (exp(s*theta/||x|| - 17)) and folded
  back into the final log via its input scale e^C; p_mem is only needed
  up to a scale.  The shift is safe for this input distribution
  (5*s_max ~ 21.9 -> max exp value ~135 < e4m3's 240) and entries below
  e4m3's subnormal floor contribute < 1e-3 relatively.
* both matmul stages run in fp8 DoubleRow perf mode (2 fp8 weights per
  PE cell, K=256 per matmul), halving PE streaming time so the tensor
  engine stays off the critical path.
* keys+vals for one 512-row window ship as a single fused [128, 12192]
  fp8 DMA (1.56 MB) from a contiguous DRAM block.
* a dummy [1,1] AllReduce in the prologue absorbs the ~11us
  collective-core startup latency that would otherwise sit on the tail,
  and the Ln activation table is preloaded for the same reason.

x ships as fp32 and is split on-device into an fp8 (hi, lo*16) pair
used as an M=2 DoubleRow stationary; the hi/lo recombination runs as a
bf16 [2,128]^T @ [1, 1/16] transpose matmul (bf16 weights take the fast
FWL load path), and the theta/||x|| scale is applied at fp32 precision
inside the Exp activation.

The tensor engine idles part of each window in this DMA-bound pipeline,
so the HAM clock gate oscillates between 1.2 and 2.4 GHz; keeping the
per-window PE work well under the DMA period (DoubleRow + bf16
transposes) makes the cold phases harmless.  Measured: ~250us of
compute per core (DMA busy ~94% of the chip-aggregate HBM roofline when
all 8 cores run together), plus ~20us collective tail, plus whatever
launch skew the runtime draws that run (0-165us observed).

Self-contained: hardcodes all shapes; imports only the system-installed
concourse stack + numpy.
"""

from contextlib import ExitStack

import ml_dtypes
import numpy as np

import concourse.bass as bass
import concourse.tile as tile
from concourse import bacc, mybir

F32 = mybir.dt.float32
BF16 = mybir.dt.bfloat16
F8 = mybir.dt.float8e4
AF = mybir.ActivationFunctionType
DR = mybir.MatmulPerfMode.DoubleRow
F8_NP = ml_dtypes.float8_e4m3  # TRN FP8_EXP4-compatible (max 240)

# Problem shapes (full)
D_FEAT = 2048
N_MEM = 200000
N_CLASSES = 1000
THETA = 5.0
N_CORES = 8

# Per-core sharding: 25000 n-rows, zero-padded to 25088 = 49*512
N_SHARD = N_MEM // N_CORES          # 25000
WIN = 512                           # n-window width (one psum bank of f32)
N_PAD = 25088                       # 49 windows * 512
N_WINDOWS = N_PAD // WIN            # 49
CHUNKS_PER_WIN = WIN // 128         # 4
FEAT_CHUNKS = D_FEAT // 128         # 16
NC_HALF = N_CLASSES // 2            # 500 (<=512 psum free-dim limit)
KEY_BYTES = FEAT_CHUNKS * WIN       # 8192 per partition per window
VAL_BYTES = CHUNKS_PER_WIN * N_CLASSES  # 4000 per partition per window
KV_BYTES = KEY_BYTES + VAL_BYTES    # 12192
C_SHIFT = 17.0                      # global exp shift; added back post-log
XLO_SCALE = 16.0                    # x-lo residual premultiplier


def build_kernel(num_devices: int = N_CORES, kv_bufs: int = 12):
    """Builds + compiles the per-core Bass program (SPMD: same program on
    every core; each core receives its own fused keys/vals shard)."""
    nc = bacc.Bacc(
        "TRN2",
        target_bir_lowering=False,
        debug=False,
        num_devices=num_devices,
    )

    x_d = nc.dram_tensor("x", [1, D_FEAT], F32, kind="ExternalInput").ap()
    # Fused keys+vals blocks, two windows per contiguous DMA (3.1 MB each;
    # per-partition runs stay fully contiguous so DMA sustains line rate):
    #   window block w at pair t=w//2, half i=w%2:
    #     kv[t, p, i*KVB + c*WIN + j]        = e4m3(keys[c*128+p, w*WIN+j])
    #     kv[t, p, i*KVB + KEYB + q*NCLS+j]  = e4m3(vals[(w*4+q)*128+p, j])
    n_pairs = (N_WINDOWS + 1) // 2
    kv_d = nc.dram_tensor(
        "kv", [n_pairs, 128, 2 * KV_BYTES], F8, kind="ExternalInput"
    ).ap()
    out_d = nc.dram_tensor("out", [1, N_CLASSES], F32, kind="ExternalOutput").ap()

    with tile.TileContext(nc) as tc, ExitStack() as ctx:
        const = ctx.enter_context(tc.tile_pool(name="const", bufs=1))
        kv_pool = ctx.enter_context(tc.tile_pool(name="kv", bufs=kv_bufs))
        s_pool = ctx.enter_context(tc.tile_pool(name="s", bufs=4))
        ss_pool = ctx.enter_context(tc.tile_pool(name="ss", bufs=4))
        psum_s = ctx.enter_context(tc.tile_pool(name="psum_s", bufs=3, space="PSUM"))
        psum_t = ctx.enter_context(tc.tile_pool(name="psum_t", bufs=3, space="PSUM"))
        psum_p = ctx.enter_context(tc.tile_pool(name="psum_p", bufs=1, space="PSUM"))
        dram = ctx.enter_context(tc.tile_pool(name="dram", bufs=1, space="DRAM"))

        # ---- prologue.  Ordering is ramp-critical: the tiny x DMA goes
        # first on the sync queue (it gates the x fp8 split), window 0's kv
        # DMA right behind it; the x hi/lo split (the only stage-1
        # dependency) is emitted before the norm/scale chain, whose products
        # (scaleB/w2row/biasC) are not needed until the first post-chain two
        # windows later.
        xt = const.tile([128, FEAT_CHUNKS], F32)
        nc.sync.dma_start(out=xt[:], in_=x_d.rearrange("a (c p) -> p (a c)", p=128))

        kv0_t = kv_pool.tile([128, KV_BYTES], F8, tag="kv")
        nc.sync.dma_start(out=kv0_t[:], in_=kv_d[0, :, 0:KV_BYTES])

        # x fp8 hi/lo split (lo premultiplied by XLO_SCALE), laid out for
        # DoubleRow: xs[p, c, m], m in {hi, lo}; inner dim padded to 16 so
        # the k-pair stride is 16B (LDWEIGHTS DoubleRow AP constraint).
        xh8 = const.tile([128, FEAT_CHUNKS], F8)
        nc.vector.tensor_copy(xh8[:], xt[:])
        xh32 = const.tile([128, FEAT_CHUNKS], F32)
        nc.vector.tensor_copy(xh32[:], xh8[:])
        xl32 = const.tile([128, FEAT_CHUNKS], F32)
        nc.vector.tensor_sub(xl32[:], xt[:], xh32[:])
        xl16 = const.tile([128, FEAT_CHUNKS], F32)
        nc.vector.tensor_scalar_mul(xl16[:], xl32[:], XLO_SCALE)
        xs = const.tile([128, FEAT_CHUNKS, 16], F8)
        nc.vector.tensor_copy(
            xs[:, :, 0:1], xh8[:].rearrange("p (c o) -> p c o", o=1)
        )
        nc.vector.tensor_copy(
            xs[:, :, 1:2], xl16[:].rearrange("p (c o) -> p c o", o=1)
        )

        ones = const.tile([128, 1], F32)
        nc.vector.memset(ones[:], 1.0)

        sq = const.tile([128, FEAT_CHUNKS], F32)
        nc.vector.tensor_mul(sq[:], xt[:], xt[:])
        sums = const.tile([128, 1], F32)
        nc.vector.tensor_reduce(
            sums[:], sq[:], axis=mybir.AxisListType.X, op=mybir.AluOpType.add
        )
        nrm2_ps = psum_t.tile([1, 1], F32, tag="ps_t")
        nc.tensor.matmul(nrm2_ps[:], lhsT=ones[:], rhs=sums[:], start=True, stop=True)
        nrm = const.tile([1, 1], F32)
        nc.scalar.sqrt(nrm[:], nrm2_ps[:])
        inv = const.tile([1, 1], F32)
        nc.vector.reciprocal(inv[:], nrm[:])
        scale = const.tile([1, 1], F32)
        nc.vector.tensor_scalar_mul(scale[:], inv[:], THETA)
        # Preload the Ln activation table now; its first use otherwise sits
        # on the tail critical path as a 1.3us ACT_TABLE_LOAD.
        ln_warm = const.tile([1, 1], F32)
        nc.scalar.activation(ln_warm[:], nrm[:], AF.Ln)
        # w2row = [1, 1/16]^T in bf16 (exact): recombines the hi/lo planes in
        # the [2,128]^T @ w2row transpose matmul.  bf16 weights get the fast
        # FWL weight-load path; fp32 would cost ~4x there.  (Built via a tiny
        # transpose matmul — engines cannot memset at a partition offset.)
        w2f = const.tile([1, 2], F32)
        nc.vector.memset(w2f[:, 0:1], 1.0)
        nc.vector.memset(w2f[:, 1:2], 1.0 / XLO_SCALE)
        onep = const.tile([1, 1], F32)
        nc.vector.memset(onep[:], 1.0)
        w2_ps = psum_t.tile([2, 1], F32, tag="ps_t")
        nc.tensor.matmul(w2_ps[:], lhsT=w2f[:], rhs=onep[:], start=True, stop=True)
        w2row = const.tile([2, 1], BF16)
        nc.vector.tensor_copy(w2row[:], w2_ps[:])
        # scaleB = theta/||x|| broadcast to [128,1]; applied at full fp32
        # precision inside the Exp activation (out = exp(in*scale + bias)).
        ones_row = const.tile([1, 128], F32)
        nc.vector.memset(ones_row[:], 1.0)
        scaleB_ps = psum_t.tile([128, 1], F32, tag="ps_t")
        nc.tensor.matmul(scaleB_ps[:], lhsT=ones_row[:], rhs=scale[:], start=True, stop=True)
        scaleB = const.tile([128, 1], F32)
        nc.vector.tensor_copy(scaleB[:], scaleB_ps[:])
        biasC = const.tile([128, 1], F32)
        nc.vector.memset(biasC[:], -C_SHIFT)

        # Warm the collective-compute cores with a tiny dummy AllReduce now:
        # the first collective pays ~11us of CC startup latency before its
        # mesh begins, which otherwise lands on the tail critical path.
        warm_sb = const.tile([1, 1], F32)
        nc.vector.memset(warm_sb[:], 0.0)
        warm_in = dram.tile([1, 1], F32)
        warm_out = dram.tile([1, 1], F32)
        nc.gpsimd.dma_start(warm_in[:], warm_sb[:])
        nc.gpsimd.collective_compute(
            "AllReduce",
            mybir.AluOpType.add,
            replica_groups=[list(range(num_devices))],
            ins=[warm_in.opt()],
            outs=[warm_out.opt()],
        )

        # ---- persistent [1, NC_HALF] accumulators (class halves)
        pp_a = psum_p.tile([1, NC_HALF], F32, tag="pp_a")
        pp_b = psum_p.tile([1, NC_HALF], F32, tag="pp_b")

        n_groups = N_WINDOWS * 2  # stage-2 accumulation steps per class half

        def emit_post(ps_s, vals, w):
            s2 = s_pool.tile([2, WIN], BF16, tag="s2")
            nc.vector.tensor_copy(s2[:], ps_s[:])
            # ss[p, q, 0] = e4m3(exp(scale*(hi + lo/16) - C)): bf16 transpose
            # matmul recombines hi/lo, then Exp applies the fp32 scale and
            # bias and writes fp8 directly (max value ~134 < e4m3's 240).
            ss = ss_pool.tile([128, CHUNKS_PER_WIN, 16], F8, tag="ss")
            for q in range(CHUNKS_PER_WIN):
                ps_t = psum_t.tile([128, 1], F32, tag="ps_t")
                nc.tensor.matmul(
                    ps_t[:],
                    lhsT=s2[:, q * 128:(q + 1) * 128],
                    rhs=w2row[:],
                    start=True,
                    stop=True,
                )
                nc.scalar.activation(
                    ss[:, q, 0:1], ps_t[:], AF.Exp, bias=biasC[:], scale=scaleB[:]
                )
            # stage 2: fp8 DoubleRow, two n-chunk pairs x two class halves
            for r in range(2):
                gc = w * 2 + r
                first = gc == 0
                last = gc == n_groups - 1
                for pp, j0 in ((pp_a, 0), (pp_b, NC_HALF)):
                    nc.tensor.matmul(
                        pp[:],
                        lhsT=ss[:, 2 * r:2 * r + 2, 0:1],
                        rhs=vals[:, 2 * r:2 * r + 2, j0:j0 + NC_HALF],
                        start=first,
                        stop=last,
                        perf_mode=DR,
                        skip_group_check=True,
                    )

        # Software-pipelined emission, depth 2: window w's post-chain
        # (transpose/exp/stage-2) is emitted after window w+2's stage-1
        # matmuls, giving the ACT/DVE exp+cast chain a full extra window
        # to complete before the PE needs its stage-2 operands.  One
        # dma_start per window: 1.56 MB transfers sustain a higher measured
        # line rate than 3.1 MB ones, and per-window completion lets each
        # stage-1 start sooner.
        pends = []
        for w in range(N_WINDOWS):
            if w == 0:
                kv_t = kv0_t  # DMA'd at the top of the prologue
            else:
                kv_t = kv_pool.tile([128, KV_BYTES], F8, tag="kv")
                nc.sync.dma_start(
                    out=kv_t[:],
                    in_=kv_d[
                        w // 2, :, (w % 2) * KV_BYTES:(w % 2 + 1) * KV_BYTES
                    ],
                )
            keys = kv_t[:, 0:KEY_BYTES].rearrange(
                "p (c j) -> p c j", c=FEAT_CHUNKS
            )
            vals = kv_t[:, KEY_BYTES:KV_BYTES].rearrange(
                "p (q j) -> p q j", q=CHUNKS_PER_WIN
            )
            # stage 1: fp8 DoubleRow, 8 matmuls of K=256 each
            ps_s = psum_s.tile([2, WIN], F32)
            for c in range(FEAT_CHUNKS // 2):
                nc.tensor.matmul(
                    ps_s[:],
                    lhsT=xs[:, 2 * c:2 * c + 2, 0:2],
                    rhs=keys[:, 2 * c:2 * c + 2, :],
                    start=(c == 0),
                    stop=(c == FEAT_CHUNKS // 2 - 1),
                    perf_mode=DR,
                    skip_group_check=True,
                )

            pends.append((ps_s, vals, w))
            if len(pends) > 2:
                emit_post(*pends.pop(0))
        for p in pends:
            emit_post(*p)

        # ---- tail: partial p = [pp_a | pp_b]; AllReduce; log; +C
        p_sb = const.tile([1, N_CLASSES], F32)
        nc.vector.tensor_copy(p_sb[:, 0:NC_HALF], pp_a[:])
        nc.vector.tensor_copy(p_sb[:, NC_HALF:N_CLASSES], pp_b[:])

        partial = dram.tile([1, N_CLASSES], F32)
        reduced = dram.tile([1, N_CLASSES], F32)
        nc.sync.dma_start(partial[:], p_sb[:])
        nc.gpsimd.collective_compute(
            "AllReduce",
            mybir.AluOpType.add,
            replica_groups=[list(range(num_devices))],
            ins=[partial.opt()],
            outs=[reduced.opt()],
        )
        # Read the reduced partial back spread over 8 partitions, and fold
        # the +C un-shift into the Ln via its input scale:
        # ln(p*e^-C * e^C) = ln(p).
        red_sb = const.tile([8, N_CLASSES // 8], F32)
        nc.sync.dma_start(
            red_sb[:], reduced[:].rearrange("a (p j) -> p (a j)", p=8)
        )
        lg = const.tile([8, N_CLASSES // 8], F32)
        nc.scalar.activation(
            lg[:], red_sb[:], AF.Ln, scale=float(np.exp(C_SHIFT))
        )
        nc.sync.dma_start(out_d.rearrange("a (p j) -> p (a j)", p=8), lg[:])

    nc.compile()
    return nc


_NC_CACHE: dict = {}


def _get_nc():
    if "nc" not in _NC_CACHE:
        _NC_CACHE["nc"] = build_kernel()
    return _NC_CACHE["nc"]


def _retile_keys(keys_shard):
    """[D_FEAT, N_PAD] e4m3 -> [N_WINDOWS, 128, KEY_BYTES] with
    out[w, p, c*WIN + j] = keys_shard[c*128 + p, w*WIN + j]."""
    v = keys_shard.reshape(FEAT_CHUNKS, 128, N_WINDOWS, WIN)
    return np.ascontiguousarray(v.transpose(2, 1, 0, 3)).reshape(
        N_WINDOWS, 128, KEY_BYTES
    )


def _retile_vals(vals_shard):
    """[N_PAD, N_CLASSES] e4m3 -> [N_WINDOWS, 128, VAL_BYTES] with
    out[w, p, q*NCLS + j] = vals_shard[(w*4 + q)*128 + p, j]."""
    v = vals_shard.reshape(N_WINDOWS, CHUNKS_PER_WIN, 128, N_CLASSES)
    return np.ascontiguousarray(v.transpose(0, 2, 1, 3)).reshape(
        N_WINDOWS, 128, VAL_BYTES
    )


def _shard_inputs(x, mem_keys, mem_vals):
    x = np.ascontiguousarray(np.asarray(x, dtype=np.float32))
    keys8 = np.asarray(mem_keys, dtype=np.float32).astype(F8_NP)
    vals8 = np.asarray(mem_vals, dtype=np.float32).astype(F8_NP)
    n_pairs = (N_WINDOWS + 1) // 2
    in_maps = []
    for i in range(N_CORES):
        lo_i, hi_i = i * N_SHARD, (i + 1) * N_SHARD
        keys_shard = np.zeros((D_FEAT, N_PAD), dtype=F8_NP)
        keys_shard[:, :N_SHARD] = keys8[:, lo_i:hi_i]
        vals_shard = np.zeros((N_PAD, N_CLASSES), dtype=F8_NP)
        vals_shard[:N_SHARD, :] = vals8[lo_i:hi_i, :]
        kv = np.concatenate(
            [_retile_keys(keys_shard), _retile_vals(vals_shard)], axis=2
        )
        # pair consecutive windows contiguously: kv2[t, p, i*KVB + b]
        kv2 = np.zeros((n_pairs, 128, 2 * KV_BYTES), dtype=F8_NP)
        kv2[: N_WINDOWS // 2] = (
            kv[: N_WINDOWS // 2 * 2]
            .reshape(N_WINDOWS // 2, 2, 128, KV_BYTES)
            .transpose(0, 2, 1, 3)
            .reshape(N_WINDOWS // 2, 128, 2 * KV_BYTES)
        )
        if N_WINDOWS % 2:
            kv2[-1, :, 0:KV_BYTES] = kv[-1]
        in_maps.append({"x": x, "kv": np.ascontiguousarray(kv2)})
    return in_maps


def run(x, mem_keys, mem_vals, trace: bool = False):
    """Runs the SPMD kernel; returns (output [1, N_CLASSES], BassKernelResults)."""
    from concourse.bass_utils import run_bass_kernel_spmd

    nc = _get_nc()
    in_maps = _shard_inputs(x, mem_keys, mem_vals)
    res = run_bass_kernel_spmd(nc, in_maps, list(range(N_CORES)), trace=trace)
    out = np.asarray(res.results[0]["out"], dtype=np.float32).reshape(1, N_CLASSES)
    return out, res


def kernel(x, mem_keys, mem_vals):
    out, _ = run(x, mem_keys, mem_vals, trace=False)
    return out
